# revision 64
# baseline (speedup 1.0000x reference)
"""Trainium2 Bass kernel for batched cross-attention with gaussian guide mask.

Reference computation (per batch b):
  Q   = query @ Wq.T                      # [Tq, A]
  att = (Q @ K.T / sqrt(A)) * guide       # guide[n] = exp(-(step-(n+1)/N)^2/TEMP)
  att = where(mask, -inf, att)
  out = softmax(att, axis=-1) @ V         # [Tq, E]

Sharding: data-parallel over batch. Core b handles batch b (B == 8 == n_cores).

Design (measured on HW, ~67us vs 80-93us for the transpose-based version):
- The attention matrix is computed ALREADY TRANSPOSED — for each n-tile,
  attT[n, t] = sum_a ksc[a, n] * qT[a, t] with the ksc n-tile as the
  stationary operand.  This removes all 128 PE transposes of the score
  matrix (and their PSUM->SBUF copies) that a forward-layout kernel needs
  to feed the AV matmul; the PE stream is pure useful streaming at the
  bf16 floor (AV 512-col matmuls at ~215ns, attT 128-col at ~56ns).
- Softmax denominators: a DVE f16 add-tree folds the 16 n-tiles of the
  exp'd scores into ssum[p, t] (max 16*e^5.5 ~ 4e3, inside f16 range) and
  ONE 1-column matmul contracts over partitions — cheaper than 16
  interleaved 1-col matmuls (~26ns of PE issue floor each).
- The guide (a pure elementwise function of the `step` input) is folded
  into the host-side K transpose/cast: ksc = K.T * guide / sqrt(A) — same
  DMA bytes, and it removes the on-device step-broadcast DMA (8.3us of
  4-byte packets!), two 1.8us iotas and four serialized activations from
  the startup critical path.
- The mask arrives host-transposed in the same (n-partition, t-free)
  layout and is applied AFTER exp by predicated-zeroing the f16 score
  tiles (half the DVE cost of a f32 pre-exp predicate).
- Softmax needs no max-subtraction (logits are O(5)); normalization is
  applied to the narrow [128, 512] AV output.  Output is stored f16 and
  upcast on host.
- Startup is DMA-fabric-bound (~330 GB/s aggregate across the 3 rings,
  ~0.7us per doorbell): the startup-critical tensors ride three rings
  concurrently in consumption order (sync: wqq+first masks+qt2/3+outs;
  scalar: ksc, qt1, V tail; gpsimd: V head, later masks), and a burst of
  dummy matmuls after the preamble barrier keeps the PE busy through the
  HAM activity window so the real matmuls start at 2.4 GHz, not 1.2 GHz.
- PSUM: attT tiles 4x[128,512]f32 (4 banks, own pool so the rotation is
  freed tile-by-tile by exp), AV 2 + qproj 1, rowsum 1 = 8 banks exactly.

Host does layout-only prep + the O(N*A) guide fold (no O(Tq*N) FLOPs):
  wqT  = [Wq.T tiles | query.T chunk 0]  (one DMA feeds the first matmuls)
  qT   = query[b].T chunks 1-3           [3*128, 8*256] f16
  ksc  = K[b].T * guide / sqrt(A)        [128, 2048]    f16
  v    = V[b] n-tile-major               [128, 16*512]  f16
  msk  = mask[b] transposed per t-tile   [1024, 2048]   u8
         (row ti*128+p, col nt*128+t  =  mask[b][ti*128+t, nt*128+p])
"""

import math

import numpy as np

import concourse.bass as bass
import concourse.mybir as mybir
import concourse.tile as tile
from concourse import bacc
from concourse.bass import ts
from concourse.bass_utils import run_bass_kernel_spmd

B, TQ, N = 8, 1024, 2048
L, A, E = 1024, 128, 512
TEMP = 0.08
P = 128
LT = L // P    # 8 l-tiles (contraction tiles of the Q projection)
TT = TQ // P   # 8 t-tiles (rows of attention, 128 at a time)
NT = N // P    # 16 n-tiles (contraction tiles of the AV matmul)
NG = 4         # n-tiles per attT psum tile ([128, 512] f32 = 1 bank)
NWARM = 46     # HAM warm-up matmuls (~4.9us of PE activity at 1.2 GHz)

F32 = mybir.dt.float32
F16 = mybir.dt.float16
U8 = mybir.dt.uint8


def build_nc():
    nc = bacc.Bacc("TRN2", target_bir_lowering=False, debug=False, enable_asserts=False, num_devices=B)

    qT = nc.dram_tensor("qT", [3 * P, LT * 256], F16, kind="ExternalInput").ap()
    kscd = nc.dram_tensor("kscd", [A, N], F16, kind="ExternalInput").ap()
    v = nc.dram_tensor("v", [P, NT * E], F16, kind="ExternalInput").ap()
    wqT = nc.dram_tensor("wqT", [P, LT * A + LT * 256], F16, kind="ExternalInput").ap()
    msk = nc.dram_tensor("msk", [TQ, N], U8, kind="ExternalInput").ap()
    out = nc.dram_tensor("out", [TQ, E], F16, kind="ExternalOutput").ap()

    with tile.TileContext(nc) as tc:
        with (
            tc.tile_pool(name="const", bufs=1) as const,
            tc.tile_pool(name="setup", bufs=1) as setup,
            tc.tile_pool(name="mpool", bufs=3) as mpool,
            tc.tile_pool(name="spool", bufs=4) as spool,
            tc.tile_pool(name="rpool", bufs=3) as rpool,
            tc.tile_pool(name="opool", bufs=3) as opool,
            tc.tile_pool(name="small", bufs=6) as small,
            tc.tile_pool(name="psA", bufs=4, space="PSUM") as psA,
            tc.tile_pool(name="psO", bufs=2, space="PSUM") as psO,
            tc.tile_pool(name="psR", bufs=1, space="PSUM") as psR,
        ):
            # ---- one-time setup ----
            # HAM warm-up: dummy matmuls on a zeroed tile keep the PE
            # busy from the preamble barrier until the first input DMA lands,
            # so the activity monitor un-throttles the clock before the real
            # matmuls start.  Results are discarded.
            junk = const.tile([P, P], F16)
            nc.vector.memset(junk, 0.0)
            for w8 in range(NWARM // 4):
                ps_warm = psA.tile([P, 4 * P], F32, tag="att", name="ps_warm")
                for j in range(4):
                    nc.tensor.matmul(
                        ps_warm[:, ts(j, P)], junk, junk, start=True, stop=True
                    )

            # The DMA fabric is a shared ~360 GB/s pool across the three
            # rings; the startup chain (wq -> qproj -> attT needs ksc; av
            # needs v) is DMA-latency-bound, so the three startup-critical
            # tensors go on three different rings concurrently and nothing
            # else is allowed in front of them.
            wqq = const.tile([P, LT * A + LT * 256], F16)
            nc.sync.dma_start(out=wqq, in_=wqT)
            wq_sb = wqq[:, : LT * A].rearrange("p (lt a) -> p lt a", lt=LT)

            # first two masks prefetched on the sync ring behind wqq, with qt
            # chunk 1 slotted BETWEEN them: mask0 and qt1 both land before
            # their consumers (pred(0) / qproj(1)) while mask1 still arrives
            # well ahead of pred(1); ksc heads the scalar ring.
            mk01 = {}
            mk0 = mpool.tile([P, N], U8, name="mk")
            nc.sync.dma_start(out=mk0, in_=msk[ts(0, P), :])
            mk01[0] = mk0

            ksc = const.tile([P, N], F16)
            nc.scalar.dma_start(out=ksc, in_=kscd)

            zeros = const.tile([P, N], F16)
            nc.vector.memset(zeros, 0.0)
            ones = const.tile([P, 1], F16)
            nc.vector.memset(ones, 1.0)

            # Q^T[a, t] = sum_l Wq[a, l] * query[t, l].
            # query.T arrives in four t-chunks so the projection (and the
            # first att tiles) start before the whole query arrives.
            QCH = TQ // 4
            qt_in = setup.tile([P, 3, LT, QCH], F16)
            qt = const.tile([P, TQ], F16)
            v_sb = const.tile([P, NT, E], F16)
            qt0_in = wqq[:, LT * A :].rearrange("p (lt t) -> p lt t", lt=LT)

            def load_qt_chunk(q):
                # chunk 1 rides the sync ring (between the first two masks),
                # chunk 2 the gpsimd ring (behind the V head, ahead of the
                # late masks), chunk 3 the sync ring — each lands ~2 tiles
                # before its projection needs it.
                eng = {1: nc.sync, 2: nc.gpsimd, 3: nc.sync}[q]
                eng.dma_start(out=qt_in[:, q - 1, :, :], in_=qT[ts(q - 1, P), :])

            def project_qt_chunk(q):
                src_q = qt0_in if q == 0 else qt_in[:, q - 1, :, :]
                ps_qt = psO.tile([P, QCH], F32, tag="qp", bufs=1, name="ps_qt")
                for lt in range(LT):
                    nc.tensor.matmul(
                        ps_qt,
                        wq_sb[:, lt, :],
                        src_q[:, lt, :],
                        start=(lt == 0),
                        stop=(lt == LT - 1),
                    )
                nc.scalar.copy(qt[:, ts(q, QCH)], ps_qt)

            # qt1 between the two mask prefetches on sync; mask1 follows.
            load_qt_chunk(1)
            mk1 = mpool.tile([P, N], U8, name="mk")
            nc.sync.dma_start(out=mk1, in_=msk[ts(1, P), :])
            mk01[1] = mk1
            # V in quarters, split across the gpsimd and scalar rings in
            # consumption order so av(0) streams behind the arriving chunks.
            for vh in range(4):
                eng = nc.gpsimd if vh < 2 else nc.scalar
                eng.dma_start(
                    out=v_sb[:, ts(vh, NT // 4), :],
                    in_=v[:, ts(vh, NT * E // 4)],
                )

            # ---- main loop: software-pipelined over 128-row tiles of Tq ----
            # Stage attT(ti): mask DMA, 16 transposed att matmuls (4 psum
            #   tiles of [128, 512] f32, one bank each).
            # Stage exp+pred(ti): 4 scalar exps PSUM -> st [128,2048] f16 SBUF
            #   interleaved with 2 half-predicates zeroing masked lanes, so
            #   the AV matmuls of the first n-tiles unblock early.
            # Stage av(ti):   16x (AV matmul [128,512] + rowsum matmul
            #   [128,1] sharing the st weight tile), both psum-accumulated.
            # Stage fin(ti):  reciprocal rowsum, normalize f16, store.
            # Emission is skewed so the PE stream alternates attT(ti+1) and
            # av(ti) with no idle gaps while scalar exp + DVE predicate of
            # tile ti run under av(ti-1)/attT(ti+1).
            stash = {}

            def stage_attT(ti):
                # qt chunk 2/3 doorbells ring just-in-time (two tiles of
                # lead); chunk 1 was issued in setup
                if ti in (2, 4):
                    load_qt_chunk(ti // 2 + 1)
                if ti < 2:
                    mk = mk01.pop(ti)
                else:
                    mk = mpool.tile([P, N], U8, name="mk")
                    nc.gpsimd.dma_start(out=mk, in_=msk[ts(ti, P), :])
                pss = []
                for g in range(NT // NG):
                    ps_att = psA.tile([P, NG * P], F32, tag="att", name="ps_att")
                    for j in range(NG):
                        nt = g * NG + j
                        nc.tensor.matmul(
                            ps_att[:, ts(j, P)],
                            ksc[:, ts(nt, P)],
                            qt[:, ts(ti, P)],
                            start=True,
                            stop=True,
                        )
                    pss.append(ps_att)
                stash[ti] = (mk, pss)

            stash_s = {}

            def stage_exp(ti):
                mk, pss = stash.pop(ti)
                s = spool.tile([P, N], F16, name="s")
                H = N // 2
                for g in range(NT // NG):
                    nc.scalar.activation(
                        out=s[:, ts(g, NG * P)],
                        in_=pss[g],
                        func=mybir.ActivationFunctionType.Exp,
                    )
                    if g % 2 == 1:
                        h = g // 2
                        nc.vector.copy_predicated(
                            out=s[:, ts(h, H)],
                            mask=mk[:, ts(h, H)],
                            data=zeros[:, ts(h, H)],
                        )
                stash_s[ti] = s

            stash_o = {}

            def stage_av(ti):
                s = stash_s.pop(ti)
                # rowsum: DVE f16 add-tree folds the 16 n-tiles of s into
                # ssum[p, t] (max value 16*e^5.5 ~ 4e3, safely inside f16),
                # then ONE 1-column matmul contracts over partitions.  This
                # keeps the softmax denominator off the PE stream (16
                # interleaved 1-col matmuls cost ~26ns of issue floor each).
                w = rpool.tile([P, 1920], F16, name="w")
                nc.vector.tensor_add(w[:, 0:512], s[:, 0:512], s[:, 512:1024])
                nc.vector.tensor_add(
                    w[:, 512:1024], s[:, 1024:1536], s[:, 1536:2048]
                )
                nc.vector.tensor_add(w[:, 1024:1536], w[:, 0:512], w[:, 512:1024])
                nc.vector.tensor_add(
                    w[:, 1536:1664], w[:, 1024:1152], w[:, 1152:1280]
                )
                nc.vector.tensor_add(
                    w[:, 1664:1792], w[:, 1280:1408], w[:, 1408:1536]
                )
                nc.vector.tensor_add(
                    w[:, 1792:1920], w[:, 1536:1664], w[:, 1664:1792]
                )
                # out[t, e] = sum_n s^T[n, t] * V[n, e].  The rowsum matmul
                # and its reciprocal run mid-chain (the add-tree is done by
                # then), so only normalize+store remain after the last AV
                # matmul — shortening the kernel tail.
                ot = psO.tile([P, E], F32, tag="pso", name="ot")
                rs = psR.tile([P, 1], F32, tag="psr", name="rs")
                rc = small.tile([P, 1], F32, name="rc")
                for nt in range(NT):
                    nc.tensor.matmul(
                        ot,
                        s[:, ts(nt, P)],
                        v_sb[:, nt, :],
                        start=(nt == 0),
                        stop=(nt == NT - 1),
                    )
                    if nt == 8:
                        nc.tensor.matmul(
                            rs, w[:, 1792:1920], ones, start=True, stop=True
                        )
                        nc.vector.reciprocal(rc, rs)
                stash_o[ti] = (ot, rc)

            def stage_fin(ti):
                ot, rc = stash_o.pop(ti)
                # normalize on the narrow output tile and store (f16)
                ob = opool.tile([P, E], F16, name="ob")
                nc.vector.tensor_scalar_mul(ob, ot, rc)
                nc.sync.dma_start(out=out[ts(ti, P), :], in_=ob)

            # skewed emission; Q-projection chunks run one iteration AHEAD of
            # the attT tiles that need them, so the PSUM->SBUF qt copy
            # overlaps an AV window instead of stalling the next attT.
            project_qt_chunk(0)
            stage_attT(0)
            stage_attT(1)
            stage_exp(0)
            stage_av(0)
            project_qt_chunk(1)
            for ti in range(2, TT):
                stage_attT(ti)
                if ti % 2 == 0 and ti // 2 + 1 < 4:
                    project_qt_chunk(ti // 2 + 1)
                stage_exp(ti - 1)
                stage_av(ti - 1)
                stage_fin(ti - 2)
            stage_exp(TT - 1)
            stage_av(TT - 1)
            stage_fin(TT - 2)
            stage_fin(TT - 1)

    nc.compile()
    return nc


def make_in_maps(query, K, V, Wq, step, mask):
    query = np.asarray(query, dtype=np.float32)
    K = np.asarray(K, dtype=np.float32)
    V = np.asarray(V, dtype=np.float32)
    Wq = np.asarray(Wq, dtype=np.float32)
    step = np.asarray(step, dtype=np.float32)
    mask = np.asarray(mask)
    if mask.dtype != np.uint8:
        mask = mask.astype(np.uint8)

    # positional gaussian guide (pure function of the scalar `step` input),
    # with the 1/sqrt(A) attention norm folded in
    chars_pos = np.arange(1, N + 1, dtype=np.float64) / N
    g = np.exp(-((float(step[0]) - chars_pos) ** 2) / TEMP) / math.sqrt(A)

    # [p][lt][a] layout: contiguous per partition row
    wq_arr = (
        Wq.T.astype(np.float16).reshape(LT, P, A).transpose(1, 0, 2).reshape(P, LT * A)
    )
    # per-batch query chunks: [4][P][LT][TQ/4], chunk-contiguous
    qt_chunks = [
        np.ascontiguousarray(
            query[b]
            .T.astype(np.float16)
            .reshape(LT, P, 4, TQ // 4)
            .transpose(2, 1, 0, 3)
        )
        for b in range(B)
    ]
    in_maps = []
    for b in range(B):
        in_maps.append(
            {
                "qT": qt_chunks[b][1:].reshape(3 * P, LT * (TQ // 4)),
                "kscd": np.ascontiguousarray(
                    (K[b].T * g[None, :]).astype(np.float16)
                ),
                "v": np.ascontiguousarray(
                    V[b]
                    .astype(np.float16)
                    .reshape(NT, P, E)
                    .transpose(1, 0, 2)
                    .reshape(P, NT * E)
                ),
                "wqT": np.ascontiguousarray(
                    np.concatenate(
                        [wq_arr, qt_chunks[b][0].reshape(P, LT * (TQ // 4))], axis=1
                    )
                ),
                # transposed per t-tile: row ti*128+p, col nt*128+t
                "msk": np.ascontiguousarray(
                    mask[b].reshape(TT, P, NT, P).transpose(0, 3, 2, 1).reshape(TQ, N)
                ),
            }
        )
    return in_maps


def kernel(query, K, V, Wq, step, mask):
    nc = build_nc()
    in_maps = make_in_maps(query, K, V, Wq, step, mask)
    res = run_bass_kernel_spmd(nc, in_maps, core_ids=list(range(B)))
    return np.stack(
        [res.results[b]["out"].astype(np.float32) for b in range(B)], axis=0
    )


if __name__ == "__main__":
    rng = np.random.default_rng(0)
    inputs = {
        "query": rng.standard_normal((B, TQ, L), dtype=np.float32),
        "K": rng.standard_normal((B, N, A), dtype=np.float32),
        "V": rng.standard_normal((B, N, E), dtype=np.float32),
        "Wq": rng.standard_normal((A, L), dtype=np.float32) / math.sqrt(L),
        "step": rng.random((1,), dtype=np.float32),
        "mask": rng.integers(0, 2, size=(B, TQ, N)) > 0,
    }
    out = kernel(**inputs)
    print(out.shape, out.dtype)


# revision 67
# speedup vs baseline: 1.0301x; 1.0301x over previous
"""Trainium2 Bass kernel for batched cross-attention with gaussian guide mask.

Reference computation (per batch b):
  Q   = query @ Wq.T                      # [Tq, A]
  att = (Q @ K.T / sqrt(A)) * guide       # guide[n] = exp(-(step-(n+1)/N)^2/TEMP)
  att = where(mask, -inf, att)
  out = softmax(att, axis=-1) @ V         # [Tq, E]

Sharding: data-parallel over batch. Core b handles batch b (B == 8 == n_cores).

Design (measured on HW, ~67us vs 80-93us for the transpose-based version):
- The attention matrix is computed ALREADY TRANSPOSED — for each n-tile,
  attT[n, t] = sum_a ksc[a, n] * qT[a, t] with the ksc n-tile as the
  stationary operand.  This removes all 128 PE transposes of the score
  matrix (and their PSUM->SBUF copies) that a forward-layout kernel needs
  to feed the AV matmul; the PE stream is pure useful streaming at the
  bf16 floor (AV 512-col matmuls at ~215ns, attT 128-col at ~56ns).
- Softmax denominators: a DVE f16 add-tree folds the 16 n-tiles of the
  exp'd scores into ssum[p, t] (max 16*e^5.5 ~ 4e3, inside f16 range) and
  ONE 1-column matmul contracts over partitions — cheaper than 16
  interleaved 1-col matmuls (~26ns of PE issue floor each).
- The guide (a pure elementwise function of the `step` input) is folded
  into the host-side K transpose/cast: ksc = K.T * guide / sqrt(A) — same
  DMA bytes, and it removes the on-device step-broadcast DMA (8.3us of
  4-byte packets!), two 1.8us iotas and four serialized activations from
  the startup critical path.
- The mask arrives host-transposed in the same (n-partition, t-free)
  layout and is applied AFTER exp by predicated-zeroing the f16 score
  tiles (half the DVE cost of a f32 pre-exp predicate).
- Softmax needs no max-subtraction (logits are O(5)); normalization is
  applied to the narrow [128, 512] AV output.  Output is stored f16 and
  upcast on host.
- Startup is DMA-fabric-bound (~330 GB/s aggregate across the 3 rings,
  ~0.7us per doorbell): the startup-critical tensors ride three rings
  concurrently in consumption order (sync: wqq+first masks+qt2/3+outs;
  scalar: ksc, qt1, V tail; gpsimd: V head, later masks), and a burst of
  dummy matmuls after the preamble barrier keeps the PE busy through the
  HAM activity window so the real matmuls start at 2.4 GHz, not 1.2 GHz.
- PSUM: attT tiles 4x[128,512]f32 (4 banks, own pool so the rotation is
  freed tile-by-tile by exp), AV 2 + qproj 1, rowsum 1 = 8 banks exactly.

Host does layout-only prep + the O(N*A) guide fold (no O(Tq*N) FLOPs):
  wqT  = [Wq.T tiles | query.T chunk 0]  (one DMA feeds the first matmuls)
  qT   = query[b].T chunks 1-3           [3*128, 8*256] f16
  ksc  = K[b].T * guide / sqrt(A)        [128, 2048]    f16
  v    = V[b] n-tile-major               [128, 16*512]  f16
  msk  = mask[b] transposed per t-tile   [1024, 2048]   u8
         (row ti*128+p, col nt*128+t  =  mask[b][ti*128+t, nt*128+p])
"""

import math

import numpy as np

import concourse.bass as bass
import concourse.mybir as mybir
import concourse.tile as tile
from concourse import bacc
from concourse.bass import ts
from concourse.bass_utils import run_bass_kernel_spmd

B, TQ, N = 8, 1024, 2048
L, A, E = 1024, 128, 512
TEMP = 0.08
P = 128
LT = L // P    # 8 l-tiles (contraction tiles of the Q projection)
TT = TQ // P   # 8 t-tiles (rows of attention, 128 at a time)
NT = N // P    # 16 n-tiles (contraction tiles of the AV matmul)
NG = 4         # n-tiles per attT psum tile ([128, 512] f32 = 1 bank)
NWARM = 46     # HAM warm-up matmuls (~4.9us of PE activity at 1.2 GHz)

F32 = mybir.dt.float32
F16 = mybir.dt.float16
U8 = mybir.dt.uint8


def build_nc():
    nc = bacc.Bacc("TRN2", target_bir_lowering=False, debug=False, enable_asserts=False, num_devices=B)

    qT = nc.dram_tensor("qT", [3 * P, LT * 256], F16, kind="ExternalInput").ap()
    kscd = nc.dram_tensor("kscd", [A, N], F16, kind="ExternalInput").ap()
    v = nc.dram_tensor("v", [P, NT * E], F16, kind="ExternalInput").ap()
    wqT = nc.dram_tensor("wqT", [P, LT * A + LT * 256], F16, kind="ExternalInput").ap()
    msk = nc.dram_tensor("msk", [TQ, N], U8, kind="ExternalInput").ap()
    out = nc.dram_tensor("out", [TQ, E], F16, kind="ExternalOutput").ap()

    with tile.TileContext(nc) as tc:
        with (
            tc.tile_pool(name="const", bufs=1) as const,
            tc.tile_pool(name="setup", bufs=1) as setup,
            tc.tile_pool(name="mpool", bufs=3) as mpool,
            tc.tile_pool(name="spool", bufs=4) as spool,
            tc.tile_pool(name="rpool", bufs=3) as rpool,
            tc.tile_pool(name="opool", bufs=3) as opool,
            tc.tile_pool(name="small", bufs=6) as small,
            tc.tile_pool(name="psA", bufs=4, space="PSUM") as psA,
            tc.tile_pool(name="psO", bufs=2, space="PSUM") as psO,
            tc.tile_pool(name="psR", bufs=1, space="PSUM") as psR,
        ):
            # ---- one-time setup ----
            # HAM warm-up: dummy matmuls on a zeroed tile keep the PE
            # busy from the preamble barrier until the first input DMA lands,
            # so the activity monitor un-throttles the clock before the real
            # matmuls start.  Results are discarded.
            junk = const.tile([P, P], F16)
            nc.vector.memset(junk, 0.0)
            for w8 in range(NWARM // 4):
                ps_warm = psA.tile([P, 4 * P], F32, tag="att", name="ps_warm")
                for j in range(4):
                    nc.tensor.matmul(
                        ps_warm[:, ts(j, P)], junk, junk, start=True, stop=True
                    )

            # The DMA fabric is a shared ~360 GB/s pool across the three
            # rings; the startup chain (wq -> qproj -> attT needs ksc; av
            # needs v) is DMA-latency-bound, so the three startup-critical
            # tensors go on three different rings concurrently and nothing
            # else is allowed in front of them.
            wqq = const.tile([P, LT * A + LT * 256], F16)
            nc.sync.dma_start(out=wqq, in_=wqT)
            wq_sb = wqq[:, : LT * A].rearrange("p (lt a) -> p lt a", lt=LT)

            # first two masks prefetched right behind wqq on the sync ring so
            # the first predicates never wait; later masks ride gpsimd; ksc
            # heads the scalar ring.
            mk01 = {}
            for mi in range(2):
                mk = mpool.tile([P, N], U8, name="mk")
                nc.sync.dma_start(out=mk, in_=msk[ts(mi, P), :])
                mk01[mi] = mk

            ksc = const.tile([P, N], F16)
            nc.scalar.dma_start(out=ksc, in_=kscd)

            zeros = const.tile([P, N], F16)
            nc.vector.memset(zeros, 0.0)
            ones = const.tile([P, 1], F16)
            nc.vector.memset(ones, 1.0)

            # Q^T[a, t] = sum_l Wq[a, l] * query[t, l].
            # query.T arrives in four t-chunks so the projection (and the
            # first att tiles) start before the whole query arrives.
            QCH = TQ // 4
            qt_in = setup.tile([P, 3, LT, QCH], F16)
            qt = const.tile([P, TQ], F16)
            v_sb = const.tile([P, NT, E], F16)
            qt0_in = wqq[:, LT * A :].rearrange("p (lt t) -> p lt t", lt=LT)

            def load_qt_chunk(q):
                # chunk 1 rides the scalar ring (behind ksc), chunk 2 the
                # gpsimd ring (behind the V head, ahead of the late masks),
                # chunk 3 the sync ring — each lands ~2 tiles before its
                # projection needs it.
                eng = {1: nc.scalar, 2: nc.gpsimd, 3: nc.sync}[q]
                eng.dma_start(out=qt_in[:, q - 1, :, :], in_=qT[ts(q - 1, P), :])

            def project_qt_chunk(q):
                src_q = qt0_in if q == 0 else qt_in[:, q - 1, :, :]
                ps_qt = psO.tile([P, QCH], F32, tag="qp", bufs=1, name="ps_qt")
                for lt in range(LT):
                    nc.tensor.matmul(
                        ps_qt,
                        wq_sb[:, lt, :],
                        src_q[:, lt, :],
                        start=(lt == 0),
                        stop=(lt == LT - 1),
                    )
                nc.scalar.copy(qt[:, ts(q, QCH)], ps_qt)

            # qt chunk 1 right after ksc on the scalar ring: its projection
            # sits directly after av(0) in the PE stream, so it must land
            # before the V tail.
            load_qt_chunk(1)
            # V in quarters, split across the gpsimd and scalar rings in
            # consumption order so av(0) streams behind the arriving chunks.
            for vh in range(4):
                eng = nc.gpsimd if vh < 2 else nc.scalar
                eng.dma_start(
                    out=v_sb[:, ts(vh, NT // 4), :],
                    in_=v[:, ts(vh, NT * E // 4)],
                )

            # ---- main loop: software-pipelined over 128-row tiles of Tq ----
            # Stage attT(ti): mask DMA, 16 transposed att matmuls (4 psum
            #   tiles of [128, 512] f32, one bank each).
            # Stage exp+pred(ti): 4 scalar exps PSUM -> st [128,2048] f16 SBUF
            #   interleaved with 2 half-predicates zeroing masked lanes, so
            #   the AV matmuls of the first n-tiles unblock early.
            # Stage av(ti):   16x (AV matmul [128,512] + rowsum matmul
            #   [128,1] sharing the st weight tile), both psum-accumulated.
            # Stage fin(ti):  reciprocal rowsum, normalize f16, store.
            # Emission is skewed so the PE stream alternates attT(ti+1) and
            # av(ti) with no idle gaps while scalar exp + DVE predicate of
            # tile ti run under av(ti-1)/attT(ti+1).
            stash = {}

            def stage_attT(ti):
                # qt chunk 2/3 doorbells ring just-in-time (two tiles of
                # lead); chunk 1 was issued in setup
                if ti in (2, 4):
                    load_qt_chunk(ti // 2 + 1)
                if ti < 2:
                    mk = mk01.pop(ti)
                else:
                    mk = mpool.tile([P, N], U8, name="mk")
                    nc.gpsimd.dma_start(out=mk, in_=msk[ts(ti, P), :])
                pss = []
                for g in range(NT // NG):
                    ps_att = psA.tile([P, NG * P], F32, tag="att", name="ps_att")
                    for j in range(NG):
                        nt = g * NG + j
                        nc.tensor.matmul(
                            ps_att[:, ts(j, P)],
                            ksc[:, ts(nt, P)],
                            qt[:, ts(ti, P)],
                            start=True,
                            stop=True,
                        )
                    pss.append(ps_att)
                stash[ti] = (mk, pss)

            stash_s = {}

            def stage_exp(ti):
                mk, pss = stash.pop(ti)
                s = spool.tile([P, N], F16, name="s")
                H = N // 2
                for g in range(NT // NG):
                    nc.scalar.activation(
                        out=s[:, ts(g, NG * P)],
                        in_=pss[g],
                        func=mybir.ActivationFunctionType.Exp,
                    )
                    if g % 2 == 1:
                        h = g // 2
                        nc.vector.copy_predicated(
                            out=s[:, ts(h, H)],
                            mask=mk[:, ts(h, H)],
                            data=zeros[:, ts(h, H)],
                        )
                stash_s[ti] = s

            stash_o = {}

            def stage_av(ti):
                s = stash_s.pop(ti)
                # rowsum: DVE f16 add-tree folds the 16 n-tiles of s into
                # ssum[p, t] (max value 16*e^5.5 ~ 4e3, safely inside f16),
                # then ONE 1-column matmul contracts over partitions.  This
                # keeps the softmax denominator off the PE stream (16
                # interleaved 1-col matmuls cost ~26ns of issue floor each).
                w = rpool.tile([P, 1920], F16, name="w")
                nc.vector.tensor_add(w[:, 0:512], s[:, 0:512], s[:, 512:1024])
                nc.vector.tensor_add(
                    w[:, 512:1024], s[:, 1024:1536], s[:, 1536:2048]
                )
                nc.vector.tensor_add(w[:, 1024:1536], w[:, 0:512], w[:, 512:1024])
                nc.vector.tensor_add(
                    w[:, 1536:1664], w[:, 1024:1152], w[:, 1152:1280]
                )
                nc.vector.tensor_add(
                    w[:, 1664:1792], w[:, 1280:1408], w[:, 1408:1536]
                )
                nc.vector.tensor_add(
                    w[:, 1792:1920], w[:, 1536:1664], w[:, 1664:1792]
                )
                # out[t, e] = sum_n s^T[n, t] * V[n, e].  The rowsum matmul
                # and its reciprocal run mid-chain (the add-tree is done by
                # then), so only normalize+store remain after the last AV
                # matmul — shortening the kernel tail.
                ot = psO.tile([P, E], F32, tag="pso", name="ot")
                rs = psR.tile([P, 1], F32, tag="psr", name="rs")
                rc = small.tile([P, 1], F32, name="rc")
                for nt in range(NT):
                    nc.tensor.matmul(
                        ot,
                        s[:, ts(nt, P)],
                        v_sb[:, nt, :],
                        start=(nt == 0),
                        stop=(nt == NT - 1),
                    )
                    if nt == 8:
                        nc.tensor.matmul(
                            rs, w[:, 1792:1920], ones, start=True, stop=True
                        )
                        nc.vector.reciprocal(rc, rs)
                stash_o[ti] = (ot, rc)

            def stage_fin(ti):
                ot, rc = stash_o.pop(ti)
                # normalize on the narrow output tile and store (f16)
                ob = opool.tile([P, E], F16, name="ob")
                nc.vector.tensor_scalar_mul(ob, ot, rc)
                nc.sync.dma_start(out=out[ts(ti, P), :], in_=ob)

            # skewed emission; Q-projection chunks run one iteration AHEAD of
            # the attT tiles that need them, so the PSUM->SBUF qt copy
            # overlaps an AV window instead of stalling the next attT.
            project_qt_chunk(0)
            stage_attT(0)
            stage_attT(1)
            stage_exp(0)
            stage_av(0)
            project_qt_chunk(1)
            for ti in range(2, TT):
                stage_attT(ti)
                if ti % 2 == 0 and ti // 2 + 1 < 4:
                    project_qt_chunk(ti // 2 + 1)
                stage_exp(ti - 1)
                stage_av(ti - 1)
                stage_fin(ti - 2)
            stage_exp(TT - 1)
            stage_av(TT - 1)
            stage_fin(TT - 2)
            stage_fin(TT - 1)

    nc.compile()
    return nc


def make_in_maps(query, K, V, Wq, step, mask):
    query = np.asarray(query, dtype=np.float32)
    K = np.asarray(K, dtype=np.float32)
    V = np.asarray(V, dtype=np.float32)
    Wq = np.asarray(Wq, dtype=np.float32)
    step = np.asarray(step, dtype=np.float32)
    mask = np.asarray(mask)
    if mask.dtype != np.uint8:
        mask = mask.astype(np.uint8)

    # positional gaussian guide (pure function of the scalar `step` input),
    # with the 1/sqrt(A) attention norm folded in
    chars_pos = np.arange(1, N + 1, dtype=np.float64) / N
    g = np.exp(-((float(step[0]) - chars_pos) ** 2) / TEMP) / math.sqrt(A)

    # [p][lt][a] layout: contiguous per partition row
    wq_arr = (
        Wq.T.astype(np.float16).reshape(LT, P, A).transpose(1, 0, 2).reshape(P, LT * A)
    )
    # per-batch query chunks: [4][P][LT][TQ/4], chunk-contiguous
    qt_chunks = [
        np.ascontiguousarray(
            query[b]
            .T.astype(np.float16)
            .reshape(LT, P, 4, TQ // 4)
            .transpose(2, 1, 0, 3)
        )
        for b in range(B)
    ]
    in_maps = []
    for b in range(B):
        in_maps.append(
            {
                "qT": qt_chunks[b][1:].reshape(3 * P, LT * (TQ // 4)),
                "kscd": np.ascontiguousarray(
                    (K[b].T * g[None, :]).astype(np.float16)
                ),
                "v": np.ascontiguousarray(
                    V[b]
                    .astype(np.float16)
                    .reshape(NT, P, E)
                    .transpose(1, 0, 2)
                    .reshape(P, NT * E)
                ),
                "wqT": np.ascontiguousarray(
                    np.concatenate(
                        [wq_arr, qt_chunks[b][0].reshape(P, LT * (TQ // 4))], axis=1
                    )
                ),
                # transposed per t-tile: row ti*128+p, col nt*128+t
                "msk": np.ascontiguousarray(
                    mask[b].reshape(TT, P, NT, P).transpose(0, 3, 2, 1).reshape(TQ, N)
                ),
            }
        )
    return in_maps


def kernel(query, K, V, Wq, step, mask):
    nc = build_nc()
    in_maps = make_in_maps(query, K, V, Wq, step, mask)
    res = run_bass_kernel_spmd(nc, in_maps, core_ids=list(range(B)))
    return np.stack(
        [res.results[b]["out"].astype(np.float32) for b in range(B)], axis=0
    )


if __name__ == "__main__":
    rng = np.random.default_rng(0)
    inputs = {
        "query": rng.standard_normal((B, TQ, L), dtype=np.float32),
        "K": rng.standard_normal((B, N, A), dtype=np.float32),
        "V": rng.standard_normal((B, N, E), dtype=np.float32),
        "Wq": rng.standard_normal((A, L), dtype=np.float32) / math.sqrt(L),
        "step": rng.random((1,), dtype=np.float32),
        "mask": rng.integers(0, 2, size=(B, TQ, N)) > 0,
    }
    out = kernel(**inputs)
    print(out.shape, out.dtype)
